# revision 39
# baseline (speedup 1.0000x reference)
import os
import sys

import numpy as np


def _ensure_path():
    try:
        import concourse.bass  # noqa: F401
        return
    except ImportError:
        pass
    for p in ("/opt/trn_rl_repo", "/root/.axon_site/_ro/trn_rl_repo"):
        if os.path.isdir(p) and p not in sys.path:
            sys.path.insert(0, p)
    import concourse.bass  # noqa: F401


LAGS = (1, 2, 3, 7, 14, 28)
MAX_LAG = 28
CTX = 168
HP = 24
HID = 512
G = 4 * HID
B = 512
NCORES = 8
BL = B // NCORES  # 64

_F32 = np.float32
_F16 = np.float16


def _gate_perm():
    # Gate-output permutation so that the four 512-wide matmul n-tiles are
    # [i0|f0], [i1|f1], [g0|o0], [g1|o1] (x0 = x[:256], x1 = x[256:]).
    # With col-tiling (tile pairs stacked on psum partitions 0:64 / 64:128)
    # the elementwise phase then runs on a folded [128, 256] layout:
    #   p = batch + 64*(hid >= 256), q = hid % 256.
    i = np.arange(0, 512)
    f = 512 + np.arange(0, 512)
    g = 1024 + np.arange(0, 512)
    o = 1536 + np.arange(0, 512)
    return np.concatenate(
        [i[:256], f[:256], i[256:], f[256:], g[:256], o[:256], g[256:], o[256:]]
    )


# ---------------------------------------------------------------------------
# Bass program construction
# ---------------------------------------------------------------------------

_BUILT = {}  # (ctx, hp) -> runner


def _build_nc(ctx, hp):
    _ensure_path()
    import concourse.bacc as bacc
    import concourse.mybir as mybir
    from concourse.tile import TileContext

    f32 = mybir.dt.float32
    f16 = mybir.dt.float16
    AF = mybir.ActivationFunctionType
    nstep = ctx + hp - 1
    hs = hp - 1
    seq_len = hs + MAX_LAG + 1  # pred slots + initial buffer

    nc = bacc.Bacc()

    # --- dram parameters (per-core shapes) ---
    d_enc = nc.declare_dram_parameter("enc_inT", [128, ctx * BL], f16, isOutput=False)
    d_w0i = nc.declare_dram_parameter("w0i", [128, G], f16, isOutput=False)
    d_w0h = nc.declare_dram_parameter("w0h", [128, 4 * G], f16, isOutput=False)
    d_w1 = nc.declare_dram_parameter("w1", [128, 8 * G], f16, isOutput=False)
    d_wh = nc.declare_dram_parameter("wh", [128, 4], f16, isOutput=False)
    d_b1a = nc.declare_dram_parameter("b1a", [128, 512], f32, isOutput=False)
    d_b1b = nc.declare_dram_parameter("b1b", [128, 512], f32, isOutput=False)
    d_bh = nc.declare_dram_parameter("bh64", [BL, 1], f32, isOutput=False)
    d_covs = nc.declare_dram_parameter("covs", [BL, max(3 * hs, 1)], f16, isOutput=False)
    d_buf0 = nc.declare_dram_parameter("buf0", [BL, MAX_LAG + 1], f32, isOutput=False)
    d_scale = nc.declare_dram_parameter("scale", [BL, 1], f32, isOutput=False)
    # identity-selector rhs tiles for transpose-as-matmul:
    #   idh[g][k, b] = 1 iff k == b + 64*g ;  idb[k, b] = 1 iff k == b
    d_idh = nc.declare_dram_parameter("idh", [128, 128], f16, isOutput=False)
    d_y = nc.declare_dram_parameter("y", [BL, nstep], f32, isOutput=True)

    with TileContext(nc) as tc:
        with (
            tc.sbuf_pool(name="state", bufs=1) as st,
            tc.sbuf_pool(name="work", bufs=2) as wk,
            tc.psum_pool(name="g0", bufs=2) as gp0,
            tc.psum_pool(name="g1", bufs=1) as gp1,
            tc.psum_pool(name="tp", bufs=1) as tp,
        ):
            # --- resident tensors ---
            enc = st.tile([128, ctx * BL], f16, name="enc")
            w0i = st.tile([128, G], f16, name="w0i")
            w0h = st.tile([128, 4 * G], f16, name="w0h")
            w1 = st.tile([128, 8 * G], f16, name="w1")
            wh = st.tile([128, 4], f16, name="wh")
            b1a = st.tile([128, 512], f32, name="b1a")
            b1b = st.tile([128, 512], f32, name="b1b")
            bh64 = st.tile([BL, 1], f32, name="bh64")
            covs = st.tile([BL, max(3 * hs, 1)], f16, name="covs")
            scale = st.tile([BL, 1], f32, name="scale")
            idh = st.tile([128, 128], f16, name="idh")
            seq = st.tile([BL, seq_len], f32, name="seq")
            xt = st.tile([128, 64], f16, name="xt")
            xtT = st.tile([128, BL], f16, name="xtT")
            y_all = st.tile([BL, nstep], f32, name="y_all")
            h1a = st.tile([128, 128], f16, name="h1a")
            h1b = st.tile([128, 128], f16, name="h1b")
            h2a = st.tile([128, 128], f16, name="h2a")
            h2b = st.tile([128, 128], f16, name="h2b")
            c1 = st.tile([128, 256], f32, name="c1")
            c2 = st.tile([128, 256], f32, name="c2")

            # split the initial loads across both DMA-capable queues so the
            # first-step matmuls aren't serialized behind the whole 3MB:
            # SP: what layer-0 step 0 needs (w0i+enc for the x-wave, then w0h);
            # Act queue: w1 (biggest, needed mid-step-0) + small tiles.
            nc.sync.dma_start(w0i[:], d_w0i[:])
            # split enc/w0h so the first waves unblock on partial data:
            # x(0)/x(1) need only the first enc columns; the l0 h-waves
            # consume w0h chunk-by-chunk in order
            nfirst = min(8 * BL, ctx * BL)
            nc.sync.dma_start(enc[:, 0:nfirst], d_enc[:, 0:nfirst])
            nc.sync.dma_start(w0h[:, 0 : 2 * G], d_w0h[:, 0 : 2 * G])
            nc.sync.dma_start(w0h[:, 2 * G : 4 * G], d_w0h[:, 2 * G : 4 * G])
            if ctx * BL > nfirst:
                nc.sync.dma_start(enc[:, nfirst:], d_enc[:, nfirst:])
            nc.scalar.dma_start(w1[:], d_w1[:])
            nc.scalar.dma_start(idh[:], d_idh[:])
            nc.scalar.dma_start(wh[:], d_wh[:])
            nc.scalar.dma_start(b1a[:], d_b1a[:])
            nc.scalar.dma_start(b1b[:], d_b1b[:])
            nc.scalar.dma_start(bh64[:], d_bh[:])
            nc.scalar.dma_start(covs[:], d_covs[:])
            nc.scalar.dma_start(scale[:], d_scale[:])
            nc.scalar.dma_start(seq[:, hs : hs + MAX_LAG + 1], d_buf0[:])

            for t_ in (h1a, h1b, h2a, h2b):
                nc.vector.memset(t_[:], 0.0)
            for t_ in (c1, c2):
                nc.vector.memset(t_[:], 0.0)
            nc.vector.memset(xt[:], 0.0)
            nc.vector.memset(xtT[:], 0.0)
            nc.vector.memset(xt[:, 10:11], 1.0)

            def h_chunks(a, b):
                return [a[:, 0:64], b[:, 0:64], a[:, 64:128], b[:, 64:128]]

            w0h_chunks = [w0h[:, k * G : k * G + G] for k in range(4)]
            w1_chunks = [w1[:, k * G : k * G + G] for k in range(8)]

            def emit_mm_waves(psA, psB, lhs_chunks, rhs_chunks, start, stop):
                """Emit col-tiled pair waves: for each chunk j, pair into
                psA[0:64]/psA[64:128] then psB likewise."""
                n = len(lhs_chunks)
                for j in range(n):
                    lhs, rhs = lhs_chunks[j], rhs_chunks[j]
                    s = start and j == 0
                    e = stop and j == n - 1
                    nc.tensor.matmul(psA[0:64, :], lhs, rhs[:, 0:512], start=s, stop=e)
                    nc.tensor.matmul(psA[64:128, :], lhs, rhs[:, 512:1024], start=s, stop=e)
                for j in range(n):
                    lhs, rhs = lhs_chunks[j], rhs_chunks[j]
                    s = start and j == 0
                    e = stop and j == n - 1
                    nc.tensor.matmul(psB[0:64, :], lhs, rhs[:, 1024:1536], start=s, stop=e)
                    nc.tensor.matmul(psB[64:128, :], lhs, rhs[:, 1536:2048], start=s, stop=e)

            def new_l0_tiles(t):
                psA = gp0.tile([128, 512], f32, tag="l0A", name=f"t{t}l0A")
                psB = gp0.tile([128, 512], f32, tag="l0B", name=f"t{t}l0B")
                return psA, psB

            def new_l1_tiles(t):
                psA = gp1.tile([128, 512], f32, tag="l1A", name=f"t{t}l1A")
                psB = gp1.tile([128, 512], f32, tag="l1B", name=f"t{t}l1B")
                return psA, psB

            def emit_elementwise(psA, psB, c_f, hf_out, tag):
                """Gate nonlinearities + cell update.  psA = [i|f] fold,
                psB = [g|o] fold.  Writes hf_out (f16 [128, 256])."""
                sif = wk.tile([128, 512], f32, tag=tag + "sif", name=tag + "sif")
                nc.scalar.activation(sif[:], psA[:], AF.Sigmoid)
                tg = wk.tile([128, 256], f32, tag=tag + "tg", name=tag + "tg")
                nc.scalar.activation(tg[:], psB[:, 0:256], AF.Tanh)
                so = wk.tile([128, 256], f32, tag=tag + "so", name=tag + "so")
                nc.scalar.activation(so[:], psB[:, 256:512], AF.Sigmoid)
                t1 = wk.tile([128, 256], f32, tag=tag + "t1", name=tag + "t1")
                nc.gpsimd.tensor_mul(t1[:], sif[:, 256:512], c_f[:])
                t2 = wk.tile([128, 256], f32, tag=tag + "t2", name=tag + "t2")
                nc.vector.tensor_mul(t2[:], sif[:, 0:256], tg[:])
                nc.gpsimd.tensor_add(c_f[:], t1[:], t2[:])
                tch = wk.tile([128, 256], f32, tag=tag + "tc", name=tag + "tc")
                nc.scalar.activation(tch[:], c_f[:], AF.Tanh)
                nc.gpsimd.tensor_mul(hf_out[:], so[:], tch[:])

            # transpose-as-matmul: one MM per (kappa, h) with the full idh rhs
            # covers hT chunks kappa and kappa+2 at once (N=128):
            #   out[rho, n] = hf[n mod 64 + 64*(n>=64), 128*kappa + 64*h + rho]
            def emit_hT(hf, tps, base, hta, htb, tag):
                for kap in (0, 1):
                    for h in (0, 1):
                        nc.tensor.matmul(
                            tps[64 * h : 64 * h + 64, base + 128 * kap : base + 128 * kap + 128],
                            hf[:, 128 * kap + 64 * h : 128 * kap + 64 * h + 64],
                            idh[:, 0:128],
                            start=True,
                            stop=True,
                        )
                nc.scalar.copy(hta[:], tps[:, base : base + 128])
                nc.vector.tensor_copy(htb[:], tps[:, base + 128 : base + 256])

            # --- software-pipelined main loop.  PE stream per step t:
            #   [hT1(t), l1h1(t), l0(t+1) h(+enc-x) waves, hT2(t), head(t),
            #    xtT(t) (dec), dec-x-wave(t+1), l1h2(t+1)]
            # so layer-0 waves of t+1 fill the layer-1 elementwise stall of t
            # and the PE never idles long enough for HAM to re-throttle.
            # step 0: h1 = h2 = 0, so the recurrent waves are exact zeros -
            # emit only the x-wave (needs just w0i+enc from DMA) and skip the
            # l1 h2-waves; l1h1(0) opens its group instead (start=(t==0)).
            psA1, psB1 = new_l0_tiles(0)
            emit_mm_waves(
                psA1, psB1, [enc[:, 0:BL]], [w0i[:]], start=True, stop=True
            )
            psA2, psB2 = new_l1_tiles(0)

            for t in range(nstep):
                # encoder x-wave for t+1: reads only static enc and writes the
                # other l0 psum buffer (its readers finished a step ago), so it
                # runs during the PE stall before hT1(t) and opens the group
                if t + 1 < min(ctx, nstep):
                    psA1n, psB1n = new_l0_tiles(t + 1)
                    emit_mm_waves(
                        psA1n, psB1n,
                        [enc[:, (t + 1) * BL : (t + 2) * BL]], [w0i[:]],
                        start=True, stop=False,
                    )

                # layer 0 elementwise
                hf1 = wk.tile([128, 256], f16, tag="hf1", name=f"t{t}hf1")
                emit_elementwise(psA1, psB1, c1, hf1, "l0")

                # h1 transpose + copies
                tps = tp.tile([128, 512], f32, tag="tps", name=f"t{t}tps")
                emit_hT(hf1, tps, 0, h1a, h1b, f"t{t}h1")

                # layer 1, input-half waves (t=0: h2-waves were skipped, so
                # this opens the accumulation group)
                emit_mm_waves(
                    psA2, psB2, h_chunks(h1a, h1b), w1_chunks[0:4],
                    start=(t == 0), stop=True,
                )

                # layer 0 h-waves for t+1 (fills the l1-elementwise stall);
                # encoder: x-wave already ran (group open), h closes the group;
                # decoder: h opens, late x-wave (after xtT) closes it
                if t + 1 < nstep:
                    if t + 1 >= ctx:
                        psA1n, psB1n = new_l0_tiles(t + 1)
                    emit_mm_waves(
                        psA1n, psB1n, h_chunks(h1a, h1b), w0h_chunks,
                        start=(t + 1 >= ctx), stop=(t + 1 < ctx),
                    )

                # layer-1 bias then elementwise
                nc.vector.tensor_add(psA2[:], psA2[:], b1a[:])
                nc.vector.tensor_add(psB2[:], psB2[:], b1b[:])
                hf2 = wk.tile([128, 256], f16, tag="hf2", name=f"t{t}hf2")
                emit_elementwise(psA2, psB2, c2, hf2, "l1")

                # h2 transpose + copies (upper half of the shared tps bank)
                emit_hT(hf2, tps, 256, h2a, h2b, f"t{t}h2")

                if t >= ctx - 1 and t < nstep - 1:
                    s = t - (ctx - 1)
                    col = hs - 1 - s
                    # covs + lag features don't depend on y(t) (lag sources
                    # were written in earlier steps) - emit them ahead of the
                    # y-add so the strict-FIFO DVE runs them during the head
                    nc.vector.tensor_copy(xt[0:BL, 1:4], covs[:, 3 * s : 3 * s + 3])
                    # lags 1,2,3 are contiguous in seq; 7,14,28 are singles
                    nc.vector.tensor_copy(xt[0:BL, 4:7], seq[0:BL, col + 1 : col + 4])
                    for jj, lag in ((3, 7), (4, 14), (5, 28)):
                        src = col + lag
                        nc.vector.tensor_copy(
                            xt[0:BL, 4 + jj : 5 + jj], seq[0:BL, src : src + 1]
                        )

                # head
                hx = tp.tile([BL, 68], f32, tag="hx", name=f"t{t}hx")
                hd = hx[:, 64:65]
                h2c = h_chunks(h2a, h2b)
                for k in range(4):
                    nc.tensor.matmul(
                        hd, h2c[k], wh[:, k : k + 1], start=(k == 0), stop=(k == 3)
                    )
                # y_all[:, t] = head + b_head
                nc.vector.tensor_scalar_add(y_all[:, t : t + 1], hd, bh64[:, 0:1])

                # layer-1 recurrent-half waves for t+1 come BEFORE the decoder
                # xt-assembly in the PE stream: they only need the h2 copies,
                # so they fill the PE stall while DVE assembles xt.
                if t + 1 < nstep:
                    psA2, psB2 = new_l1_tiles(t + 1)
                    emit_mm_waves(
                        psA2, psB2, h_chunks(h2a, h2b), w1_chunks[4:8],
                        start=True, stop=False,
                    )

                if t >= ctx - 1 and t < nstep - 1:
                    s = t - (ctx - 1)  # decode step that CONSUMES this pred
                    col = hs - 1 - s
                    nc.vector.tensor_copy(seq[0:BL, col : col + 1], y_all[:, t : t + 1])
                    nc.vector.tensor_copy(xt[0:BL, 0:1], y_all[:, t : t + 1])
                    # xtT = xt.T via identity matmul (stays in 128x64 tile mode)
                    xps = hx[:, 0:64]
                    nc.tensor.matmul(
                        xps, xt[:, 0:64], idh[:, 0:64], start=True, stop=True
                    )
                    nc.scalar.copy(xtT[0:11, 0:BL], xps[0:11, 0:BL])

                if t + 1 < nstep:
                    if t + 1 >= ctx:
                        emit_mm_waves(
                            psA1n, psB1n, [xtT[:]], [w0i[:]], start=False, stop=True
                        )
                    psA1, psB1 = psA1n, psB1n

            nc.vector.tensor_scalar_mul(y_all[:], y_all[:], scale[:, 0:1])
            nc.sync.dma_start(d_y[:], y_all[:])

    nc.finalize()
    return nc


# ---------------------------------------------------------------------------
# Persistent PJRT runner (mirrors bass2jax.run_bass_via_pjrt, but cached so
# repeated calls do not re-trace / re-compile)
# ---------------------------------------------------------------------------


def _make_runner(nc):
    _ensure_path()
    import jax
    from jax.experimental.shard_map import shard_map
    from jax.sharding import Mesh, PartitionSpec

    import concourse.mybir as mybir
    from concourse import bass2jax

    bass2jax.install_neuronx_cc_hook()

    partition_name = nc.partition_id_tensor.name if nc.partition_id_tensor else None
    in_names, out_names, out_avals, zero_shapes = [], [], [], []
    for alloc in nc.m.functions[0].allocations:
        if not isinstance(alloc, mybir.MemoryLocationSet):
            continue
        name = alloc.memorylocations[0].name
        if alloc.kind == "ExternalInput":
            if name != partition_name:
                in_names.append(name)
        elif alloc.kind == "ExternalOutput":
            out_names.append(name)
            shape = tuple(alloc.tensor_shape)
            dtype = mybir.dt.np(alloc.dtype)
            out_avals.append(jax.core.ShapedArray(shape, dtype))
            zero_shapes.append((shape, dtype))
    n_params = len(in_names)
    n_outs = len(out_names)
    all_in = list(in_names) + list(out_names)
    if partition_name is not None:
        all_in.append(partition_name)
    all_in = tuple(all_in)

    def _body(*args):
        operands = list(args)
        if partition_name is not None:
            operands.append(bass2jax.partition_id_tensor())
        outs = bass2jax._bass_exec_p.bind(
            *operands,
            out_avals=tuple(out_avals),
            in_names=all_in,
            out_names=tuple(out_names),
            lowering_input_output_aliases=(),
            sim_require_finite=True,
            sim_require_nnan=True,
            nc=nc,
        )
        return tuple(outs)

    devices = jax.devices()[:NCORES]
    assert len(devices) == NCORES, f"need {NCORES} devices, got {len(jax.devices())}"
    mesh = Mesh(np.asarray(devices), ("core",))
    in_specs = (PartitionSpec("core"),) * (n_params + n_outs)
    out_specs = (PartitionSpec("core"),) * n_outs
    donate = tuple(range(n_params, n_params + n_outs))
    sharded = jax.jit(
        shard_map(_body, mesh=mesh, in_specs=in_specs, out_specs=out_specs, check_rep=False),
        donate_argnums=donate,
        keep_unused=True,
    )

    from jax.sharding import NamedSharding

    sharding = NamedSharding(mesh, PartitionSpec("core"))

    def prepare(in_maps):
        """device_put the concatenated inputs once; reuse across timed calls."""
        concat_in = [
            np.concatenate([np.asarray(in_maps[c][nm]) for c in range(NCORES)], axis=0)
            for nm in in_names
        ]
        return [jax.device_put(a, sharding) for a in concat_in]

    def run_prepared(dev_in):
        concat_zeros = [
            jax.device_put(np.zeros((NCORES * s[0],) + s[1:], d), sharding)
            for (s, d) in zero_shapes
        ]
        out_arrs = sharded(*dev_in, *concat_zeros)
        jax.block_until_ready(out_arrs)
        return out_arrs

    def make_zeros():
        return [
            jax.device_put(np.zeros((NCORES * s[0],) + s[1:], d), sharding)
            for (s, d) in zero_shapes
        ]

    def dispatch(dev_in, zeros):
        return sharded(*dev_in, *zeros)

    def run(in_maps):
        out_arrs = run_prepared(prepare(in_maps))
        outs = []
        for c in range(NCORES):
            outs.append(
                {
                    nm: np.asarray(out_arrs[i]).reshape((NCORES,) + zero_shapes[i][0])[c]
                    for i, nm in enumerate(out_names)
                }
            )
        return outs

    run.prepare = prepare
    run.run_prepared = run_prepared
    run.make_zeros = make_zeros
    run.dispatch = dispatch
    return run


def _get_runner(ctx, hp):
    key = (ctx, hp)
    if key not in _BUILT:
        nc = _build_nc(ctx, hp)
        _BUILT[key] = _make_runner(nc)
    return _BUILT[key]


# ---------------------------------------------------------------------------
# Host-side prep + full model entry
# ---------------------------------------------------------------------------


def _prep_in_maps(X, pad_mask, hp, ctx, W_ih0, W_hh0, b0, W_ih1, W_hh1, b1, W_head, b_head):
    f32 = _F32
    f16 = _F16
    X = np.asarray(X, f32).copy()
    pad_mask = np.asarray(pad_mask)
    B_, L_, _ = X.shape
    hs = hp - 1
    X[:, L_ - hs :, 0] = 0.0
    past = X[:, : L_ - hs, 0][:, ::-1]  # [B, MAX_LAG+ctx] newest-first
    Xs = X[:, MAX_LAG:]  # [B, ctx+hs, 3]
    m = pad_mask[:, MAX_LAG:][:, :ctx].astype(f32)
    scale = (np.abs(Xs[:, :ctx, 0]) * m).sum(1) / np.maximum(m.sum(1), 1.0)
    scale = np.maximum(scale, 1e-3).astype(f32)  # [B]
    pastn = (past / scale[:, None]).astype(f32)
    logs = np.log(scale)
    tgt = Xs[:, :, 0] / scale[:, None]

    idx = (ctx - 1 - np.arange(ctx))[:, None] + np.asarray(LAGS)[None, :]
    lags = pastn[:, idx]  # [B, ctx, 6]
    enc = np.concatenate(
        [
            tgt[:, :ctx, None],
            Xs[:, :ctx, 1:3],
            np.broadcast_to(logs[:, None, None], (B_, ctx, 1)),
            lags,
            np.ones((B_, ctx, 1), f32),
        ],
        axis=2,
    ).astype(f32)  # [B, ctx, 11]
    covs = np.concatenate(
        [Xs[:, ctx:, 1:3], np.broadcast_to(logs[:, None, None], (B_, hs, 1))], axis=2
    ).astype(f32)  # [B, hs, 3]
    buf0 = pastn[:, : MAX_LAG + 1]

    perm = _gate_perm()
    W_ih0 = np.asarray(W_ih0, f32)[perm]
    W_hh0 = np.asarray(W_hh0, f32)[perm]
    b0p = np.asarray(b0, f32)[perm]
    W_ih1 = np.asarray(W_ih1, f32)[perm]
    W_hh1 = np.asarray(W_hh1, f32)[perm]
    b1p = np.asarray(b1, f32)[perm]
    W_head = np.asarray(W_head, f32)
    b_head = np.asarray(b_head, f32)

    w0i_small = np.concatenate([W_ih0.T, b0p[None, :]], 0)  # [11, G]
    w0i = np.zeros((128, G), f16)
    w0i[0:11] = w0i_small.astype(f16)
    W0hT = W_hh0.T  # [512, G]
    w0h = np.ascontiguousarray(
        np.concatenate([W0hT[128 * k : 128 * (k + 1)] for k in range(4)], 1)
    ).astype(f16)
    W1T = np.concatenate([W_ih1.T, W_hh1.T], 0)  # [1024, G]
    w1 = np.ascontiguousarray(
        np.concatenate([W1T[128 * k : 128 * (k + 1)] for k in range(8)], 1)
    ).astype(f16)
    b1a = np.empty((128, 512), f32)
    b1a[0:64] = b1p[0:512]
    b1a[64:128] = b1p[512:1024]
    b1b = np.empty((128, 512), f32)
    b1b[0:64] = b1p[1024:1536]
    b1b[64:128] = b1p[1536:2048]
    wh = np.stack([W_head[128 * k : 128 * (k + 1), 0] for k in range(4)], 1).astype(f16)
    bh64 = np.full((BL, 1), float(b_head[0]), f32)

    # identity-selector tiles for transpose-as-matmul:
    #   idh[:, 0:64]  : rhs[k', b] = 1 iff k' == b       (hf rows 0:64, xtT)
    #   idh[:, 64:128]: rhs[k', b] = 1 iff k' == b + 64  (hf rows 64:128)
    idh = np.zeros((128, 128), f16)
    idh[np.arange(64), np.arange(64)] = 1.0
    idh[64 + np.arange(64), 64 + np.arange(64)] = 1.0

    in_maps = []
    for c in range(NCORES):
        sl = slice(c * BL, (c + 1) * BL)
        enc_inT = np.zeros((128, ctx * BL), f16)
        enc_inT[0:11] = (
            enc[sl].transpose(2, 1, 0).reshape(11, ctx * BL).astype(f16)
        )
        in_maps.append(
            {
                "enc_inT": enc_inT,
                "w0i": w0i,
                "w0h": w0h,
                "w1": w1,
                "b1a": b1a,
                "b1b": b1b,
                "wh": np.ascontiguousarray(wh),
                "bh64": bh64,
                "covs": np.ascontiguousarray(covs[sl].reshape(BL, max(3 * hs, 1))).astype(f16),
                "buf0": np.ascontiguousarray(buf0[sl]),
                "scale": np.ascontiguousarray(scale[sl, None]),
                "idh": idh,
            }
        )
    return in_maps, scale


def run_model(X, pad_mask, H, context_length, W_ih0, W_hh0, b0, W_ih1, W_hh1, b1, W_head, b_head):
    hp = int(H)
    ctx = int(context_length)
    in_maps, _ = _prep_in_maps(
        X, pad_mask, hp, ctx, W_ih0, W_hh0, b0, W_ih1, W_hh1, b1, W_head, b_head
    )
    run = _get_runner(ctx, hp)
    outs = run(in_maps)
    y = np.concatenate([outs[c]["y"] for c in range(NCORES)], axis=0)  # [B, nstep]
    return y[:, :, None].astype(_F32)


def kernel(**inputs):
    return run_model(
        inputs["X"],
        inputs["pad_mask"],
        inputs["H"],
        inputs["context_length"],
        inputs["W_ih0"],
        inputs["W_hh0"],
        inputs["b0"],
        inputs["W_ih1"],
        inputs["W_hh1"],
        inputs["b1"],
        inputs["W_head"],
        inputs["b_head"],
    )


# revision 40
# speedup vs baseline: 1.1597x; 1.1597x over previous
import os
import sys

import numpy as np


def _ensure_path():
    try:
        import concourse.bass  # noqa: F401
        return
    except ImportError:
        pass
    for p in ("/opt/trn_rl_repo", "/root/.axon_site/_ro/trn_rl_repo"):
        if os.path.isdir(p) and p not in sys.path:
            sys.path.insert(0, p)
    import concourse.bass  # noqa: F401


LAGS = (1, 2, 3, 7, 14, 28)
MAX_LAG = 28
CTX = 168
HP = 24
HID = 512
G = 4 * HID
B = 512
NCORES = 8
BL = B // NCORES  # 64

_F32 = np.float32
_F16 = np.float16


def _gate_perm():
    # Gate-output permutation so that the four 512-wide matmul n-tiles are
    # [i0|f0], [i1|f1], [g0|o0], [g1|o1] (x0 = x[:256], x1 = x[256:]).
    # With col-tiling (tile pairs stacked on psum partitions 0:64 / 64:128)
    # the elementwise phase then runs on a folded [128, 256] layout:
    #   p = batch + 64*(hid >= 256), q = hid % 256.
    i = np.arange(0, 512)
    f = 512 + np.arange(0, 512)
    g = 1024 + np.arange(0, 512)
    o = 1536 + np.arange(0, 512)
    return np.concatenate(
        [i[:256], f[:256], i[256:], f[256:], g[:256], o[:256], g[256:], o[256:]]
    )


# ---------------------------------------------------------------------------
# Bass program construction
# ---------------------------------------------------------------------------

_BUILT = {}  # (ctx, hp) -> runner


def _build_nc(ctx, hp):
    _ensure_path()
    import concourse.bacc as bacc
    import concourse.mybir as mybir
    from concourse.tile import TileContext

    f32 = mybir.dt.float32
    f16 = mybir.dt.float16
    AF = mybir.ActivationFunctionType
    nstep = ctx + hp - 1
    hs = hp - 1
    seq_len = hs + MAX_LAG + 1  # pred slots + initial buffer

    nc = bacc.Bacc()

    # --- dram parameters (per-core shapes) ---
    d_enc = nc.declare_dram_parameter("enc_inT", [128, ctx * BL], f16, isOutput=False)
    d_w0i = nc.declare_dram_parameter("w0i", [128, G], f16, isOutput=False)
    d_w0h = nc.declare_dram_parameter("w0h", [128, 4 * G], f16, isOutput=False)
    d_w1 = nc.declare_dram_parameter("w1", [128, 8 * G], f16, isOutput=False)
    d_wh = nc.declare_dram_parameter("wh", [128, 4], f16, isOutput=False)
    d_b1a = nc.declare_dram_parameter("b1a", [128, 512], f32, isOutput=False)
    d_b1b = nc.declare_dram_parameter("b1b", [128, 512], f32, isOutput=False)
    d_bh = nc.declare_dram_parameter("bh64", [BL, 1], f32, isOutput=False)
    d_covs = nc.declare_dram_parameter("covs", [BL, max(3 * hs, 1)], f16, isOutput=False)
    d_buf0 = nc.declare_dram_parameter("buf0", [BL, MAX_LAG + 1], f32, isOutput=False)
    d_scale = nc.declare_dram_parameter("scale", [BL, 1], f32, isOutput=False)
    # identity-selector rhs tiles for transpose-as-matmul:
    #   idh[g][k, b] = 1 iff k == b + 64*g ;  idb[k, b] = 1 iff k == b
    d_idh = nc.declare_dram_parameter("idh", [128, 128], f16, isOutput=False)
    d_y = nc.declare_dram_parameter("y", [BL, nstep], f32, isOutput=True)

    with TileContext(nc) as tc:
        with (
            tc.sbuf_pool(name="state", bufs=1) as st,
            tc.sbuf_pool(name="work", bufs=2) as wk,
            tc.psum_pool(name="g0", bufs=2) as gp0,
            tc.psum_pool(name="g1", bufs=1) as gp1,
            tc.psum_pool(name="tp", bufs=1) as tp,
        ):
            # --- resident tensors ---
            enc = st.tile([128, ctx * BL], f16, name="enc")
            w0i = st.tile([128, G], f16, name="w0i")
            w0h = st.tile([128, 4 * G], f16, name="w0h")
            w1 = st.tile([128, 8 * G], f16, name="w1")
            wh = st.tile([128, 4], f16, name="wh")
            b1a = st.tile([128, 512], f32, name="b1a")
            b1b = st.tile([128, 512], f32, name="b1b")
            bh64 = st.tile([BL, 1], f32, name="bh64")
            covs = st.tile([BL, max(3 * hs, 1)], f16, name="covs")
            scale = st.tile([BL, 1], f32, name="scale")
            idh = st.tile([128, 128], f16, name="idh")
            seq = st.tile([BL, seq_len], f32, name="seq")
            xt = st.tile([128, 64], f16, name="xt")
            xtT = st.tile([128, BL], f16, name="xtT")
            y_all = st.tile([BL, nstep], f32, name="y_all")
            h1a = st.tile([128, 128], f16, name="h1a")
            h1b = st.tile([128, 128], f16, name="h1b")
            h2a = st.tile([128, 128], f16, name="h2a")
            h2b = st.tile([128, 128], f16, name="h2b")
            c1 = st.tile([128, 256], f32, name="c1")
            c2 = st.tile([128, 256], f32, name="c2")

            # split the initial loads across both DMA-capable queues so the
            # first-step matmuls aren't serialized behind the whole 3MB:
            # SP: what layer-0 step 0 needs (w0i+enc for the x-wave, then w0h);
            # Act queue: w1 (biggest, needed mid-step-0) + small tiles.
            nc.sync.dma_start(w0i[:], d_w0i[:])
            # split enc/w0h so the first waves unblock on partial data:
            # x(0)/x(1) need only the first enc columns; the l0 h-waves
            # consume w0h chunk-by-chunk in order
            nfirst = min(8 * BL, ctx * BL)
            nc.sync.dma_start(enc[:, 0:nfirst], d_enc[:, 0:nfirst])
            nc.sync.dma_start(w0h[:, 0 : 2 * G], d_w0h[:, 0 : 2 * G])
            nc.sync.dma_start(w0h[:, 2 * G : 4 * G], d_w0h[:, 2 * G : 4 * G])
            if ctx * BL > nfirst:
                nc.sync.dma_start(enc[:, nfirst:], d_enc[:, nfirst:])
            nc.scalar.dma_start(w1[:], d_w1[:])
            nc.scalar.dma_start(idh[:], d_idh[:])
            nc.scalar.dma_start(wh[:], d_wh[:])
            nc.scalar.dma_start(b1a[:], d_b1a[:])
            nc.scalar.dma_start(b1b[:], d_b1b[:])
            nc.scalar.dma_start(bh64[:], d_bh[:])
            nc.scalar.dma_start(covs[:], d_covs[:])
            nc.scalar.dma_start(scale[:], d_scale[:])
            nc.scalar.dma_start(seq[:, hs : hs + MAX_LAG + 1], d_buf0[:])

            for t_ in (h1a, h1b, h2a, h2b):
                nc.vector.memset(t_[:], 0.0)
            for t_ in (c1, c2):
                nc.vector.memset(t_[:], 0.0)
            nc.vector.memset(xt[:], 0.0)
            nc.vector.memset(xtT[:], 0.0)
            nc.vector.memset(xt[:, 10:11], 1.0)

            def h_chunks(a, b):
                return [a[:, 0:64], b[:, 0:64], a[:, 64:128], b[:, 64:128]]

            w0h_chunks = [w0h[:, k * G : k * G + G] for k in range(4)]
            w1_chunks = [w1[:, k * G : k * G + G] for k in range(8)]

            def emit_mm_waves(psA, psB, lhs_chunks, rhs_chunks, start, stop):
                """Emit col-tiled pair waves: for each chunk j, pair into
                psA[0:64]/psA[64:128] then psB likewise."""
                n = len(lhs_chunks)
                for j in range(n):
                    lhs, rhs = lhs_chunks[j], rhs_chunks[j]
                    s = start and j == 0
                    e = stop and j == n - 1
                    nc.tensor.matmul(psA[0:64, :], lhs, rhs[:, 0:512], start=s, stop=e)
                    nc.tensor.matmul(psA[64:128, :], lhs, rhs[:, 512:1024], start=s, stop=e)
                for j in range(n):
                    lhs, rhs = lhs_chunks[j], rhs_chunks[j]
                    s = start and j == 0
                    e = stop and j == n - 1
                    nc.tensor.matmul(psB[0:64, :], lhs, rhs[:, 1024:1536], start=s, stop=e)
                    nc.tensor.matmul(psB[64:128, :], lhs, rhs[:, 1536:2048], start=s, stop=e)

            def new_l0_tiles(t):
                psA = gp0.tile([128, 512], f32, tag="l0A", name=f"t{t}l0A")
                psB = gp0.tile([128, 512], f32, tag="l0B", name=f"t{t}l0B")
                return psA, psB

            def new_l1_tiles(t):
                psA = gp1.tile([128, 512], f32, tag="l1A", name=f"t{t}l1A")
                psB = gp1.tile([128, 512], f32, tag="l1B", name=f"t{t}l1B")
                return psA, psB

            def emit_elementwise(psA, psB, c_f, hf_out, tag):
                """Gate nonlinearities + cell update.  psA = [i|f] fold,
                psB = [g|o] fold.  Writes hf_out (f16 [128, 256])."""
                sif = wk.tile([128, 512], f32, tag=tag + "sif", name=tag + "sif")
                nc.scalar.activation(sif[:], psA[:], AF.Sigmoid)
                tg = wk.tile([128, 256], f32, tag=tag + "tg", name=tag + "tg")
                nc.scalar.activation(tg[:], psB[:, 0:256], AF.Tanh)
                so = wk.tile([128, 256], f32, tag=tag + "so", name=tag + "so")
                nc.scalar.activation(so[:], psB[:, 256:512], AF.Sigmoid)
                t1 = wk.tile([128, 256], f32, tag=tag + "t1", name=tag + "t1")
                nc.gpsimd.tensor_mul(t1[:], sif[:, 256:512], c_f[:])
                t2 = wk.tile([128, 256], f32, tag=tag + "t2", name=tag + "t2")
                nc.vector.tensor_mul(t2[:], sif[:, 0:256], tg[:])
                nc.gpsimd.tensor_add(c_f[:], t1[:], t2[:])
                tch = wk.tile([128, 256], f32, tag=tag + "tc", name=tag + "tc")
                nc.scalar.activation(tch[:], c_f[:], AF.Tanh)
                nc.gpsimd.tensor_mul(hf_out[:], so[:], tch[:])

            # transpose-as-matmul: one MM per (kappa, h) with the full idh rhs
            # covers hT chunks kappa and kappa+2 at once (N=128):
            #   out[rho, n] = hf[n mod 64 + 64*(n>=64), 128*kappa + 64*h + rho]
            def emit_hT(hf, tps, base, hta, htb, tag):
                for kap in (0, 1):
                    for h in (0, 1):
                        nc.tensor.matmul(
                            tps[64 * h : 64 * h + 64, base + 128 * kap : base + 128 * kap + 128],
                            hf[:, 128 * kap + 64 * h : 128 * kap + 64 * h + 64],
                            idh[:, 0:128],
                            start=True,
                            stop=True,
                        )
                nc.scalar.copy(hta[:], tps[:, base : base + 128])
                nc.vector.tensor_copy(htb[:], tps[:, base + 128 : base + 256])

            # --- software-pipelined main loop.  PE stream per step t:
            #   [hT1(t), l1h1(t), l0(t+1) h(+enc-x) waves, hT2(t), head(t),
            #    xtT(t) (dec), dec-x-wave(t+1), l1h2(t+1)]
            # so layer-0 waves of t+1 fill the layer-1 elementwise stall of t
            # and the PE never idles long enough for HAM to re-throttle.
            # step 0: h1 = h2 = 0, so the recurrent waves are exact zeros -
            # emit only the x-wave (needs just w0i+enc from DMA) and skip the
            # l1 h2-waves; l1h1(0) opens its group instead (start=(t==0)).
            psA1, psB1 = new_l0_tiles(0)
            emit_mm_waves(
                psA1, psB1, [enc[:, 0:BL]], [w0i[:]], start=True, stop=True
            )
            psA2, psB2 = new_l1_tiles(0)

            for t in range(nstep):
                # encoder x-wave for t+1: reads only static enc and writes the
                # other l0 psum buffer (its readers finished a step ago), so it
                # runs during the PE stall before hT1(t) and opens the group
                if t + 1 < min(ctx, nstep):
                    psA1n, psB1n = new_l0_tiles(t + 1)
                    emit_mm_waves(
                        psA1n, psB1n,
                        [enc[:, (t + 1) * BL : (t + 2) * BL]], [w0i[:]],
                        start=True, stop=False,
                    )

                # layer 0 elementwise
                hf1 = wk.tile([128, 256], f16, tag="hf1", name=f"t{t}hf1")
                emit_elementwise(psA1, psB1, c1, hf1, "l0")

                # h1 transpose + copies
                tps = tp.tile([128, 512], f32, tag="tps", name=f"t{t}tps")
                emit_hT(hf1, tps, 0, h1a, h1b, f"t{t}h1")

                # layer 1, input-half waves (t=0: h2-waves were skipped, so
                # this opens the accumulation group)
                emit_mm_waves(
                    psA2, psB2, h_chunks(h1a, h1b), w1_chunks[0:4],
                    start=(t == 0), stop=True,
                )

                # layer 0 h-waves for t+1 (fills the l1-elementwise stall);
                # encoder: x-wave already ran (group open), h closes the group;
                # decoder: h opens, late x-wave (after xtT) closes it
                if t + 1 < nstep:
                    if t + 1 >= ctx:
                        psA1n, psB1n = new_l0_tiles(t + 1)
                    emit_mm_waves(
                        psA1n, psB1n, h_chunks(h1a, h1b), w0h_chunks,
                        start=(t + 1 >= ctx), stop=(t + 1 < ctx),
                    )

                # layer-1 bias then elementwise
                nc.vector.tensor_add(psA2[:], psA2[:], b1a[:])
                nc.vector.tensor_add(psB2[:], psB2[:], b1b[:])
                hf2 = wk.tile([128, 256], f16, tag="hf2", name=f"t{t}hf2")
                emit_elementwise(psA2, psB2, c2, hf2, "l1")

                # h2 transpose + copies (upper half of the shared tps bank)
                emit_hT(hf2, tps, 256, h2a, h2b, f"t{t}h2")

                if t >= ctx - 1 and t < nstep - 1:
                    s = t - (ctx - 1)
                    col = hs - 1 - s
                    # covs + lag features don't depend on y(t) (lag sources
                    # were written in earlier steps) - emit them ahead of the
                    # y-add so the strict-FIFO DVE runs them during the head
                    nc.vector.tensor_copy(xt[0:BL, 1:4], covs[:, 3 * s : 3 * s + 3])
                    # lags 1,2,3 are contiguous in seq; 7,14,28 are singles
                    nc.vector.tensor_copy(xt[0:BL, 4:7], seq[0:BL, col + 1 : col + 4])
                    for jj, lag in ((3, 7), (4, 14), (5, 28)):
                        src = col + lag
                        nc.vector.tensor_copy(
                            xt[0:BL, 4 + jj : 5 + jj], seq[0:BL, src : src + 1]
                        )

                # head
                hx = tp.tile([BL, 68], f32, tag="hx", name=f"t{t}hx")
                hd = hx[:, 64:65]
                h2c = h_chunks(h2a, h2b)
                for k in range(4):
                    nc.tensor.matmul(
                        hd, h2c[k], wh[:, k : k + 1], start=(k == 0), stop=(k == 3)
                    )
                # y_all[:, t] = head + b_head
                nc.vector.tensor_scalar_add(y_all[:, t : t + 1], hd, bh64[:, 0:1])

                # layer-1 recurrent-half waves for t+1 come BEFORE the decoder
                # xt-assembly in the PE stream: they only need the h2 copies,
                # so they fill the PE stall while DVE assembles xt.
                if t + 1 < nstep:
                    psA2, psB2 = new_l1_tiles(t + 1)
                    emit_mm_waves(
                        psA2, psB2, h_chunks(h2a, h2b), w1_chunks[4:8],
                        start=True, stop=False,
                    )

                if t >= ctx - 1 and t < nstep - 1:
                    s = t - (ctx - 1)  # decode step that CONSUMES this pred
                    col = hs - 1 - s
                    # xt[0] first: xps (PE) waits on it, while seq[col] isn't
                    # read until next step's lag copies - park that on Pool
                    nc.vector.tensor_copy(xt[0:BL, 0:1], y_all[:, t : t + 1])
                    nc.gpsimd.tensor_copy(seq[0:BL, col : col + 1], y_all[:, t : t + 1])
                    # xtT = xt.T via identity matmul (stays in 128x64 tile mode)
                    xps = hx[:, 0:64]
                    nc.tensor.matmul(
                        xps, xt[:, 0:64], idh[:, 0:64], start=True, stop=True
                    )
                    nc.scalar.copy(xtT[0:11, 0:BL], xps[0:11, 0:BL])

                if t + 1 < nstep:
                    if t + 1 >= ctx:
                        emit_mm_waves(
                            psA1n, psB1n, [xtT[:]], [w0i[:]], start=False, stop=True
                        )
                    psA1, psB1 = psA1n, psB1n

            nc.vector.tensor_scalar_mul(y_all[:], y_all[:], scale[:, 0:1])
            nc.sync.dma_start(d_y[:], y_all[:])

    nc.finalize()
    return nc


# ---------------------------------------------------------------------------
# Persistent PJRT runner (mirrors bass2jax.run_bass_via_pjrt, but cached so
# repeated calls do not re-trace / re-compile)
# ---------------------------------------------------------------------------


def _make_runner(nc):
    _ensure_path()
    import jax
    from jax.experimental.shard_map import shard_map
    from jax.sharding import Mesh, PartitionSpec

    import concourse.mybir as mybir
    from concourse import bass2jax

    bass2jax.install_neuronx_cc_hook()

    partition_name = nc.partition_id_tensor.name if nc.partition_id_tensor else None
    in_names, out_names, out_avals, zero_shapes = [], [], [], []
    for alloc in nc.m.functions[0].allocations:
        if not isinstance(alloc, mybir.MemoryLocationSet):
            continue
        name = alloc.memorylocations[0].name
        if alloc.kind == "ExternalInput":
            if name != partition_name:
                in_names.append(name)
        elif alloc.kind == "ExternalOutput":
            out_names.append(name)
            shape = tuple(alloc.tensor_shape)
            dtype = mybir.dt.np(alloc.dtype)
            out_avals.append(jax.core.ShapedArray(shape, dtype))
            zero_shapes.append((shape, dtype))
    n_params = len(in_names)
    n_outs = len(out_names)
    all_in = list(in_names) + list(out_names)
    if partition_name is not None:
        all_in.append(partition_name)
    all_in = tuple(all_in)

    def _body(*args):
        operands = list(args)
        if partition_name is not None:
            operands.append(bass2jax.partition_id_tensor())
        outs = bass2jax._bass_exec_p.bind(
            *operands,
            out_avals=tuple(out_avals),
            in_names=all_in,
            out_names=tuple(out_names),
            lowering_input_output_aliases=(),
            sim_require_finite=True,
            sim_require_nnan=True,
            nc=nc,
        )
        return tuple(outs)

    devices = jax.devices()[:NCORES]
    assert len(devices) == NCORES, f"need {NCORES} devices, got {len(jax.devices())}"
    mesh = Mesh(np.asarray(devices), ("core",))
    in_specs = (PartitionSpec("core"),) * (n_params + n_outs)
    out_specs = (PartitionSpec("core"),) * n_outs
    donate = tuple(range(n_params, n_params + n_outs))
    sharded = jax.jit(
        shard_map(_body, mesh=mesh, in_specs=in_specs, out_specs=out_specs, check_rep=False),
        donate_argnums=donate,
        keep_unused=True,
    )

    from jax.sharding import NamedSharding

    sharding = NamedSharding(mesh, PartitionSpec("core"))

    def prepare(in_maps):
        """device_put the concatenated inputs once; reuse across timed calls."""
        concat_in = [
            np.concatenate([np.asarray(in_maps[c][nm]) for c in range(NCORES)], axis=0)
            for nm in in_names
        ]
        return [jax.device_put(a, sharding) for a in concat_in]

    def run_prepared(dev_in):
        concat_zeros = [
            jax.device_put(np.zeros((NCORES * s[0],) + s[1:], d), sharding)
            for (s, d) in zero_shapes
        ]
        out_arrs = sharded(*dev_in, *concat_zeros)
        jax.block_until_ready(out_arrs)
        return out_arrs

    def make_zeros():
        return [
            jax.device_put(np.zeros((NCORES * s[0],) + s[1:], d), sharding)
            for (s, d) in zero_shapes
        ]

    def dispatch(dev_in, zeros):
        return sharded(*dev_in, *zeros)

    def run(in_maps):
        out_arrs = run_prepared(prepare(in_maps))
        outs = []
        for c in range(NCORES):
            outs.append(
                {
                    nm: np.asarray(out_arrs[i]).reshape((NCORES,) + zero_shapes[i][0])[c]
                    for i, nm in enumerate(out_names)
                }
            )
        return outs

    run.prepare = prepare
    run.run_prepared = run_prepared
    run.make_zeros = make_zeros
    run.dispatch = dispatch
    return run


def _get_runner(ctx, hp):
    key = (ctx, hp)
    if key not in _BUILT:
        nc = _build_nc(ctx, hp)
        _BUILT[key] = _make_runner(nc)
    return _BUILT[key]


# ---------------------------------------------------------------------------
# Host-side prep + full model entry
# ---------------------------------------------------------------------------


def _prep_in_maps(X, pad_mask, hp, ctx, W_ih0, W_hh0, b0, W_ih1, W_hh1, b1, W_head, b_head):
    f32 = _F32
    f16 = _F16
    X = np.asarray(X, f32).copy()
    pad_mask = np.asarray(pad_mask)
    B_, L_, _ = X.shape
    hs = hp - 1
    X[:, L_ - hs :, 0] = 0.0
    past = X[:, : L_ - hs, 0][:, ::-1]  # [B, MAX_LAG+ctx] newest-first
    Xs = X[:, MAX_LAG:]  # [B, ctx+hs, 3]
    m = pad_mask[:, MAX_LAG:][:, :ctx].astype(f32)
    scale = (np.abs(Xs[:, :ctx, 0]) * m).sum(1) / np.maximum(m.sum(1), 1.0)
    scale = np.maximum(scale, 1e-3).astype(f32)  # [B]
    pastn = (past / scale[:, None]).astype(f32)
    logs = np.log(scale)
    tgt = Xs[:, :, 0] / scale[:, None]

    idx = (ctx - 1 - np.arange(ctx))[:, None] + np.asarray(LAGS)[None, :]
    lags = pastn[:, idx]  # [B, ctx, 6]
    enc = np.concatenate(
        [
            tgt[:, :ctx, None],
            Xs[:, :ctx, 1:3],
            np.broadcast_to(logs[:, None, None], (B_, ctx, 1)),
            lags,
            np.ones((B_, ctx, 1), f32),
        ],
        axis=2,
    ).astype(f32)  # [B, ctx, 11]
    covs = np.concatenate(
        [Xs[:, ctx:, 1:3], np.broadcast_to(logs[:, None, None], (B_, hs, 1))], axis=2
    ).astype(f32)  # [B, hs, 3]
    buf0 = pastn[:, : MAX_LAG + 1]

    perm = _gate_perm()
    W_ih0 = np.asarray(W_ih0, f32)[perm]
    W_hh0 = np.asarray(W_hh0, f32)[perm]
    b0p = np.asarray(b0, f32)[perm]
    W_ih1 = np.asarray(W_ih1, f32)[perm]
    W_hh1 = np.asarray(W_hh1, f32)[perm]
    b1p = np.asarray(b1, f32)[perm]
    W_head = np.asarray(W_head, f32)
    b_head = np.asarray(b_head, f32)

    w0i_small = np.concatenate([W_ih0.T, b0p[None, :]], 0)  # [11, G]
    w0i = np.zeros((128, G), f16)
    w0i[0:11] = w0i_small.astype(f16)
    W0hT = W_hh0.T  # [512, G]
    w0h = np.ascontiguousarray(
        np.concatenate([W0hT[128 * k : 128 * (k + 1)] for k in range(4)], 1)
    ).astype(f16)
    W1T = np.concatenate([W_ih1.T, W_hh1.T], 0)  # [1024, G]
    w1 = np.ascontiguousarray(
        np.concatenate([W1T[128 * k : 128 * (k + 1)] for k in range(8)], 1)
    ).astype(f16)
    b1a = np.empty((128, 512), f32)
    b1a[0:64] = b1p[0:512]
    b1a[64:128] = b1p[512:1024]
    b1b = np.empty((128, 512), f32)
    b1b[0:64] = b1p[1024:1536]
    b1b[64:128] = b1p[1536:2048]
    wh = np.stack([W_head[128 * k : 128 * (k + 1), 0] for k in range(4)], 1).astype(f16)
    bh64 = np.full((BL, 1), float(b_head[0]), f32)

    # identity-selector tiles for transpose-as-matmul:
    #   idh[:, 0:64]  : rhs[k', b] = 1 iff k' == b       (hf rows 0:64, xtT)
    #   idh[:, 64:128]: rhs[k', b] = 1 iff k' == b + 64  (hf rows 64:128)
    idh = np.zeros((128, 128), f16)
    idh[np.arange(64), np.arange(64)] = 1.0
    idh[64 + np.arange(64), 64 + np.arange(64)] = 1.0

    in_maps = []
    for c in range(NCORES):
        sl = slice(c * BL, (c + 1) * BL)
        enc_inT = np.zeros((128, ctx * BL), f16)
        enc_inT[0:11] = (
            enc[sl].transpose(2, 1, 0).reshape(11, ctx * BL).astype(f16)
        )
        in_maps.append(
            {
                "enc_inT": enc_inT,
                "w0i": w0i,
                "w0h": w0h,
                "w1": w1,
                "b1a": b1a,
                "b1b": b1b,
                "wh": np.ascontiguousarray(wh),
                "bh64": bh64,
                "covs": np.ascontiguousarray(covs[sl].reshape(BL, max(3 * hs, 1))).astype(f16),
                "buf0": np.ascontiguousarray(buf0[sl]),
                "scale": np.ascontiguousarray(scale[sl, None]),
                "idh": idh,
            }
        )
    return in_maps, scale


def run_model(X, pad_mask, H, context_length, W_ih0, W_hh0, b0, W_ih1, W_hh1, b1, W_head, b_head):
    hp = int(H)
    ctx = int(context_length)
    in_maps, _ = _prep_in_maps(
        X, pad_mask, hp, ctx, W_ih0, W_hh0, b0, W_ih1, W_hh1, b1, W_head, b_head
    )
    run = _get_runner(ctx, hp)
    outs = run(in_maps)
    y = np.concatenate([outs[c]["y"] for c in range(NCORES)], axis=0)  # [B, nstep]
    return y[:, :, None].astype(_F32)


def kernel(**inputs):
    return run_model(
        inputs["X"],
        inputs["pad_mask"],
        inputs["H"],
        inputs["context_length"],
        inputs["W_ih0"],
        inputs["W_hh0"],
        inputs["b0"],
        inputs["W_ih1"],
        inputs["W_hh1"],
        inputs["b1"],
        inputs["W_head"],
        inputs["b_head"],
    )
